# revision 8
# baseline (speedup 1.0000x reference)
"""Trainium2 Bass kernel for DeepSet segment-reduce (natural-order design).

Key idea: segments are contiguous element ranges, so the per-segment max
becomes two masked segmented running-max scans (fwd + bwd) on the DVE
(`tensor_tensor_scan` with op0=add/op1=max and -BIG resets at segment
boundaries); max(fwd, bwd) broadcasts each segment's max to every element
in place. All compute stays in natural element order:

  - no host-side gather/scatter/layout at all (the big win: the baseline
    spent ~2s/call in single-core numpy reordering);
  - input transpose ([rows,64] -> [64,cols]) done on-device via PE
    transposes;
  - the last MLP layer uses the activation tile as the *stationary*
    matmul operand so the output lands element-major [rows,128] and DMAs
    straight to HBM in the reference layout -- the 8 per-core outputs
    concatenate to the final (1M,128) array with zero host reshuffling.

Sharding: elements 0..1M split into 8 equal 125k slices (core boundaries
mid-segment are fine: each core gets a +-HC halo of neighbor rows and the
scans recover full-segment maxes locally). Masks are csr-derived [1,E]
fp16 rows, DMA-broadcast to 128 partitions on device.

Self-contained: no reads of reference.py / spec.json.
"""
import zlib
import numpy as np

import concourse.bass as bass
import concourse.mybir as mybir
import concourse.tile as tile
from concourse import bacc
from concourse.bass_utils import run_bass_kernel_spmd

N = 1_000_000
N_CORES = 8
OWN = N // N_CORES            # owned elements per core
D_IN = 64
D_OUT = 128
ALPHA = 0.2
NEG = -60000.0                # segment-reset additive mask (fp16-safe)
C = 2500                      # owned columns per chunk
NCH = OWN // C                # chunks per core

F16 = mybir.dt.float16
F32 = mybir.dt.float32
PR = mybir.ActivationFunctionType.Prelu
CP = mybir.ActivationFunctionType.Copy
MAX = mybir.AluOpType.max
ADD = mybir.AluOpType.add


def _pick_halo(lmax):
    """Chunk halo H >= lmax with C + 2H a multiple of 256 (transpose pairs)."""
    h = 158  # 2500 + 2*158 = 2816 = 22*128
    while h < lmax:
        h += 128
    return h


# ----------------------------------------------------------------------------
# Device program
# ----------------------------------------------------------------------------

def build_nc(H, HC, loop_n=1):
    EIN = OWN + 2 * HC
    NCH = OWN // C            # chunks per core
    CH = C + 2 * H            # processed columns per chunk (mult of 256)
    NT = CH // 128            # 128-row tiles per chunk
    ET = 125                  # element tile for the final layer
    NE = C // ET              # 20 element tiles per chunk
    assert CH % 256 == 0 and C % ET == 0 and OWN % C == 0

    nc = bacc.Bacc("TRN2", target_bir_lowering=False, debug=False)

    xin = nc.declare_dram_parameter("xin", [EIN, D_IN], F16, isOutput=False)
    mfp = nc.declare_dram_parameter("mf", [1, EIN], F16, isOutput=False)
    mbp = nc.declare_dram_parameter("mb", [1, EIN], F16, isOutput=False)
    out = nc.declare_dram_parameter("out", [OWN, D_OUT], F32, isOutput=True)
    wnames = ["w11", "w12", "w21", "w22", "w31a", "w31b", "w32"]
    wdims = [D_IN, D_OUT, D_OUT, D_OUT, D_OUT, D_OUT, D_OUT]
    wp = {n: nc.declare_dram_parameter(n, [k, D_OUT], F16, isOutput=False)
          for n, k in zip(wnames, wdims)}
    bnames = ["b11", "b12", "b21", "b22", "b31"]
    bp = {n: nc.declare_dram_parameter(n, [D_OUT, 1], F32, isOutput=False)
          for n in bnames}
    b32p = nc.declare_dram_parameter("b32r", [1, D_OUT], F16, isOutput=False)
    idp = nc.declare_dram_parameter("ident", [128, 128], F16, isOutput=False)

    with tile.TileContext(nc) as tc:
        with (
            tc.tile_pool(name="wpool", bufs=1) as wpool,
            tc.tile_pool(name="xpool", bufs=3) as xpool,
            tc.tile_pool(name="mpool", bufs=3) as mpool,
            tc.tile_pool(name="apool", bufs=2) as apool,
            tc.tile_pool(name="opool", bufs=3) as opool,
            tc.tile_pool(name="pst", bufs=2, space="PSUM") as pst,
            tc.tile_pool(name="psa", bufs=4, space="PSUM") as psa,
            tc.tile_pool(name="pso", bufs=2, space="PSUM") as psoo,
        ):
            wt = {}
            for n, k in zip(wnames, wdims):
                wt[n] = wpool.tile([k, D_OUT], F16, tag=f"w_{n}", name=f"w_{n}")
                nc.gpsimd.dma_start(wt[n][:], wp[n][:])
            bt = {}
            for n in bnames:
                bt[n] = wpool.tile([D_OUT, 1], F32, tag=f"b_{n}", name=f"b_{n}")
                nc.gpsimd.dma_start(bt[n][:], bp[n][:])
            b32t = wpool.tile([1, D_OUT], F16, tag="b32t", name="b32t")
            nc.gpsimd.dma_start(b32t[:], b32p[:])
            idt = wpool.tile([128, 128], F16, tag="idt", name="idt")
            nc.gpsimd.dma_start(idt[:], idp[:])
            ones = wpool.tile([1, ET], F16, tag="ones", name="ones")
            nc.vector.memset(ones[:], 1.0)

            import contextlib
            loop_ctx = (tc.For_i(0, loop_n, 1) if loop_n > 1
                        else contextlib.nullcontext())
            with loop_ctx:
                for k in range(NCH):
                    base = HC - H + k * C        # xin row of chunk col 0

                    # ---- load x rows, PE-transpose to xT [64, CH]
                    xraw = xpool.tile([128, NT, D_IN], F16, tag="xraw")
                    nc.sync.dma_start(
                        xraw[:],
                        xin[base:base + CH, :].rearrange(
                            "(t p) f -> p t f", p=128, t=NT))
                    xT = xpool.tile([D_IN, CH], F16, tag="xT")
                    for j in range(NT // 2):
                        psx = pst.tile([128, 128], F16, tag="psx")
                        nc.tensor.transpose(
                            psx[:], xraw[:, 2 * j:2 * j + 2, :].rearrange(
                                "p t f -> p (t f)"), idt[:])
                        nc.vector.tensor_copy(
                            xT[:, 256 * j:256 * j + 128], psx[:D_IN, :])
                        nc.vector.tensor_copy(
                            xT[:, 256 * j + 128:256 * j + 256], psx[D_IN:, :])

                    # ---- masks (DMA broadcast [1,CH] -> [128,CH])
                    mft = mpool.tile([128, CH], F16, tag="mft")
                    nc.sync.dma_start(
                        mft[:], mfp[:, base:base + CH].broadcast_to([128, CH]))
                    mbt = mpool.tile([128, CH], F16, tag="mbt")
                    nc.sync.dma_start(
                        mbt[:], mbp[:, base:base + CH].broadcast_to([128, CH]))

                    # ---- L1 + L2 over full chunk (halo included)
                    a2 = apool.tile([D_OUT, CH], F16, tag="a2")
                    for off in range(0, CH, 512):
                        w = min(512, CH - off)
                        u1 = psa.tile([D_OUT, 512], F32, tag="u")
                        nc.tensor.matmul(u1[:, :w], wt["w11"][:],
                                         xT[:, off:off + w],
                                         start=True, stop=True)
                        a1 = xpool.tile([D_OUT, 512], F16, tag="a1")
                        nc.scalar.activation(a1[:, :w], u1[:, :w], PR,
                                             bias=bt["b11"][:], scale=1.0,
                                             alpha=ALPHA)
                        u2 = psa.tile([D_OUT, 512], F32, tag="u")
                        nc.tensor.matmul(u2[:, :w], wt["w12"][:], a1[:, :w],
                                         start=True, stop=True)
                        nc.scalar.activation(a2[:, off:off + w], u2[:, :w], PR,
                                             bias=bt["b12"][:], scale=1.0,
                                             alpha=ALPHA)

                    # ---- segmented max: fwd + bwd scans, pooled broadcast
                    fwd = apool.tile([D_OUT, CH], F16, tag="fwd")
                    nc.vector.tensor_tensor_scan(
                        fwd[:], mft[:], a2[:], NEG, op0=ADD, op1=MAX)
                    bwd = apool.tile([D_OUT, CH], F16, tag="bwd")
                    nc.vector.tensor_tensor_scan(
                        bwd[:, ::-1], mbt[:, ::-1], a2[:, ::-1], NEG,
                        op0=ADD, op1=MAX)
                    pooled = apool.tile([D_OUT, C], F16, tag="pooled")
                    nc.vector.tensor_max(
                        pooled[:], fwd[:, H:H + C], bwd[:, H:H + C])

                    # ---- set MLP on pooled (per-element broadcast already)
                    a4 = apool.tile([D_OUT, C], F16, tag="a4")
                    a5 = apool.tile([D_OUT, C], F16, tag="a5")
                    for off in range(0, C, 512):
                        w = min(512, C - off)
                        u3 = psa.tile([D_OUT, 512], F32, tag="u")
                        nc.tensor.matmul(u3[:, :w], wt["w21"][:],
                                         pooled[:, off:off + w],
                                         start=True, stop=True)
                        a3 = xpool.tile([D_OUT, 512], F16, tag="a3")
                        nc.scalar.activation(a3[:, :w], u3[:, :w], PR,
                                             bias=bt["b21"][:], scale=1.0,
                                             alpha=ALPHA)
                        u4 = psa.tile([D_OUT, 512], F32, tag="u")
                        nc.tensor.matmul(u4[:, :w], wt["w22"][:], a3[:, :w],
                                         start=True, stop=True)
                        nc.scalar.activation(a4[:, off:off + w], u4[:, :w], PR,
                                             bias=bt["b22"][:], scale=1.0,
                                             alpha=ALPHA)

                        # ---- L31: concat fusion via two accumulating matmuls
                        u5 = psa.tile([D_OUT, 512], F32, tag="u")
                        nc.tensor.matmul(u5[:, :w], wt["w31a"][:],
                                         a2[:, H + off:H + off + w],
                                         start=True, stop=False)
                        nc.tensor.matmul(u5[:, :w], wt["w31b"][:],
                                         a4[:, off:off + w],
                                         start=False, stop=True)
                        nc.scalar.activation(a5[:, off:off + w], u5[:, :w], PR,
                                             bias=bt["b31"][:], scale=1.0,
                                             alpha=ALPHA)

                    # ---- L4: stationary-swap -> element-major out + DMA
                    outsb = opool.tile([ET, NE, D_OUT], F32, tag="outsb")
                    for b in range(NE // 4):
                        po = psoo.tile([ET, 4, D_OUT], F32, tag="po")
                        for t in range(4):
                            e0 = (4 * b + t) * ET
                            nc.tensor.matmul(po[:, t, :],
                                             a5[:, e0:e0 + ET], wt["w32"][:],
                                             start=True, stop=False)
                            nc.tensor.matmul(po[:, t, :], ones[:], b32t[:],
                                             start=False, stop=True)
                        nc.scalar.activation(outsb[:, 4 * b:4 * b + 4, :],
                                             po[:], PR, bias=0.0, scale=1.0,
                                             alpha=ALPHA)
                    nc.sync.dma_start(
                        out[k * C:(k + 1) * C, :].rearrange(
                            "(t p) f -> p t f", p=ET, t=NE), outsb[:])

    nc.finalize()
    return nc


# ----------------------------------------------------------------------------
# Host side
# ----------------------------------------------------------------------------

_NC_CACHE = {}
_MASK_CACHE = {}
_XBUF_CACHE = {}
_IDENT = np.eye(128, dtype=np.float16)


def _layout(csr_idx):
    csr = np.ascontiguousarray(np.asarray(csr_idx, dtype=np.int64))
    key = (zlib.crc32(csr.tobytes()), csr.shape[0])
    hit = _MASK_CACHE.get(key)
    if hit is not None:
        return hit
    assert csr[0] == 0 and csr[-1] == N
    lmax = int(np.diff(csr).max())
    H = _pick_halo(lmax)
    HC = max(512, H)
    glen = N + 2 * HC
    mf = np.zeros(glen, np.float16)
    mb = np.zeros(glen, np.float16)
    mf[:HC] = NEG
    mf[HC + N:] = NEG
    mb[:HC] = NEG
    mb[HC + N:] = NEG
    mf[HC + csr[:-1]] = NEG                       # segment starts
    mb[np.clip(HC + csr[1:] - 1, 0, glen - 1)] = NEG  # segment last elements
    hit = dict(H=H, HC=HC, mf=mf, mb=mb)
    _MASK_CACHE[key] = hit
    return hit


def prepare(x, csr_idx, w11, s11, b11, w12, s12, b12,
            w21, s21, b21, w22, s22, b22,
            w31, s31, b31, w32, s32, b32, loop_n=1):
    x = np.asarray(x)
    lay = _layout(csr_idx)
    H, HC = lay["H"], lay["HC"]
    EIN = OWN + 2 * HC

    key = (H, HC, loop_n)
    if key not in _NC_CACHE:
        _NC_CACHE[key] = build_nc(H, HC, loop_n=loop_n)
    nc = _NC_CACHE[key]

    # padded fp16 copy of x (cast-assign writes straight into the pad buffer)
    xb = _XBUF_CACHE.get(HC)
    if xb is None:
        xb = np.zeros((N + 2 * HC, D_IN), np.float16)
        _XBUF_CACHE[HC] = xb
    xb[HC:HC + N] = x

    def wprep(w, s):
        return (np.asarray(w) * np.asarray(s)[None, :]).astype(np.float16)

    w31f = wprep(w31, s31)
    params = {
        "w11": wprep(w11, s11), "w12": wprep(w12, s12),
        "w21": wprep(w21, s21), "w22": wprep(w22, s22),
        "w31a": np.ascontiguousarray(w31f[:D_OUT]),
        "w31b": np.ascontiguousarray(w31f[D_OUT:]),
        "w32": wprep(w32, s32),
        "b11": np.asarray(b11, np.float32).reshape(D_OUT, 1),
        "b12": np.asarray(b12, np.float32).reshape(D_OUT, 1),
        "b21": np.asarray(b21, np.float32).reshape(D_OUT, 1),
        "b22": np.asarray(b22, np.float32).reshape(D_OUT, 1),
        "b31": np.asarray(b31, np.float32).reshape(D_OUT, 1),
        "b32r": np.asarray(b32, np.float16).reshape(1, D_OUT),
        "ident": _IDENT,
    }
    in_maps = []
    for c in range(N_CORES):
        lo = c * OWN
        in_maps.append({
            "xin": xb[lo:lo + EIN],
            "mf": lay["mf"][lo:lo + EIN].reshape(1, EIN),
            "mb": lay["mb"][lo:lo + EIN].reshape(1, EIN),
            **params,
        })
    return nc, in_maps, lay


def run_device(nc, in_maps):
    return run_bass_kernel_spmd(nc, in_maps, list(range(N_CORES)))


def postprocess(res, lay, n):
    views = [res.results[c]["out"] for c in range(N_CORES)]
    # the per-core results are views into one (N, D_OUT) host buffer the
    # runtime already assembled -- recover it zero-copy when possible
    b = views[0]
    while getattr(b, "base", None) is not None:
        b = b.base
    try:
        full = b.reshape(N, D_OUT) if b.size == N * D_OUT else None
    except Exception:
        full = None
    if full is not None and full.dtype == np.float32:
        from numpy.lib.array_utils import byte_bounds
        lo0 = byte_bounds(full)[0]
        ok = all(
            byte_bounds(views[c])[0] == lo0 + c * OWN * D_OUT * 4
            and views[c].shape == (OWN, D_OUT)
            for c in range(N_CORES))
        if ok:
            return full
    return np.concatenate(views, axis=0)


def kernel(x, csr_idx, **kw):
    x = np.asarray(x)
    nc, in_maps, lay = prepare(x, csr_idx, **kw)
    res = run_device(nc, in_maps)
    return postprocess(res, lay, x.shape[0])


# revision 34
# speedup vs baseline: 1.1544x; 1.1544x over previous
"""Trainium2 Bass kernel for DeepSet segment-reduce (natural-order design).

Key idea: segments are contiguous element ranges, so the per-segment max
becomes two masked segmented running-max scans (fwd + bwd) on the DVE
(`tensor_tensor_scan` with op0=add/op1=max and -BIG resets at segment
boundaries); max(fwd, bwd) broadcasts each segment's max to every element
in place. All compute stays in natural element order:

  - no host-side gather/scatter/layout at all (the big win: the baseline
    spent ~2s/call in single-core numpy reordering);
  - input transpose ([rows,64] -> [64,cols]) done on-device via PE
    transposes;
  - the last MLP layer uses the activation tile as the *stationary*
    matmul operand so the output lands element-major [rows,128] and DMAs
    straight to HBM in the reference layout -- the 8 per-core outputs
    concatenate to the final (1M,128) array with zero host reshuffling.

Sharding: elements 0..1M split into 8 equal 125k slices (core boundaries
mid-segment are fine: each core gets a +-HC halo of neighbor rows and the
scans recover full-segment maxes locally). Masks are csr-derived [1,E]
fp16 rows, DMA-broadcast to 128 partitions on device.

Self-contained: no reads of reference.py / spec.json.
"""
import zlib
import numpy as np

import concourse.bass as bass
import concourse.mybir as mybir
import concourse.tile as tile
from concourse import bacc
from concourse.bass_utils import run_bass_kernel_spmd

N = 1_000_000
N_CORES = 8
OWN = N // N_CORES            # owned elements per core
D_IN = 64
D_OUT = 128
ALPHA = 0.2
NEG = -60000.0                # segment-reset additive mask (fp16-safe)
C = 2500                      # owned columns per chunk
NCH = OWN // C                # chunks per core

F16 = mybir.dt.float16
F32 = mybir.dt.float32
PR = mybir.ActivationFunctionType.Prelu
CP = mybir.ActivationFunctionType.Copy
MAX = mybir.AluOpType.max
ADD = mybir.AluOpType.add


def _pick_halo(lmax):
    """Chunk halo H >= lmax with C + 2H a multiple of 256 (transpose pairs)."""
    h = 158  # 2500 + 2*158 = 2816 = 22*128
    while h < lmax:
        h += 128
    return h


# ----------------------------------------------------------------------------
# Device program
# ----------------------------------------------------------------------------

# engine assignment knobs (tuned via TimelineSim)
CFG = dict(copies="dve", act2="scalar", act31="dve", abufs=3, pipe=1,
           odma="sync")


def build_nc(H, HC, loop_n=1, cfg=None):
    cfg = dict(CFG if cfg is None else cfg)
    EIN = OWN + 2 * HC
    NCH = OWN // C            # chunks per core
    CH = C + 2 * H            # processed columns per chunk (mult of 256)
    NT = CH // 128            # 128-row tiles per chunk
    ET = 125                  # element tile for the final layer
    NE = C // ET              # 20 element tiles per chunk
    assert CH % 256 == 0 and C % ET == 0 and OWN % C == 0

    nc = bacc.Bacc("TRN2", target_bir_lowering=False, debug=False)

    xin = nc.declare_dram_parameter("xin", [EIN, D_IN], F16, isOutput=False)
    mfp = nc.declare_dram_parameter("mf", [1, EIN + 1], F16, isOutput=False)
    out = nc.declare_dram_parameter("out", [OWN, D_OUT], F32, isOutput=True)
    wnames = ["w11", "w12", "w21", "w22", "w31a", "w31b", "w32"]
    wdims = [D_IN, D_OUT, D_OUT, D_OUT, D_OUT, D_OUT, D_OUT]
    wp = {n: nc.declare_dram_parameter(n, [k, D_OUT], F16, isOutput=False)
          for n, k in zip(wnames, wdims)}
    bnames = ["b11", "b12", "b21", "b22", "b31"]
    bp = {n: nc.declare_dram_parameter(n, [D_OUT, 1], F32, isOutput=False)
          for n in bnames}
    b32p = nc.declare_dram_parameter("b32r", [1, D_OUT], F16, isOutput=False)
    brp = {n: nc.declare_dram_parameter(n + "r", [1, D_OUT], F16,
                                        isOutput=False)
           for n in ("b21", "b22", "b31")}
    idp = nc.declare_dram_parameter("ident", [128, 128], F16, isOutput=False)

    with tile.TileContext(nc) as tc:
        with (
            tc.tile_pool(name="wpool", bufs=1) as wpool,
            tc.tile_pool(name="xpool", bufs=cfg.get("xbufs", 3)) as xpool,
            tc.tile_pool(name="mpool", bufs=cfg.get("mbufs", 2)) as mpool,
            tc.tile_pool(name="apool", bufs=cfg["abufs"]) as apool,
            tc.tile_pool(name="opool", bufs=cfg.get("obufs", 3)) as opool,
            tc.tile_pool(name="pst", bufs=cfg.get("tbufs", 2),
                         space="PSUM") as pst,
            tc.tile_pool(name="psf", bufs=cfg.get("fbufs", 2),
                         space="PSUM") as psf,
            tc.tile_pool(name="psb", bufs=cfg.get("bbufs", 2),
                         space="PSUM") as psb,
            tc.tile_pool(name="pso", bufs=cfg.get("pobufs", 2),
                         space="PSUM") as psoo,
        ):
            wt = {}
            for n, k in zip(wnames, wdims):
                wt[n] = wpool.tile([k, D_OUT], F16, tag=f"w_{n}", name=f"w_{n}")
                nc.gpsimd.dma_start(wt[n][:], wp[n][:])
            bt = {}
            for n in bnames:
                bt[n] = wpool.tile([D_OUT, 1], F32, tag=f"b_{n}", name=f"b_{n}")
                nc.gpsimd.dma_start(bt[n][:], bp[n][:])
            b32t = wpool.tile([1, D_OUT], F16, tag="b32t", name="b32t")
            nc.gpsimd.dma_start(b32t[:], b32p[:])
            brt = {}
            for n in ("b21", "b22", "b31"):
                brt[n] = wpool.tile([1, D_OUT], F16, tag=f"br_{n}",
                                    name=f"br_{n}")
                nc.gpsimd.dma_start(brt[n][:], brp[n][:])
            idt = wpool.tile([128, 128], F16, tag="idt", name="idt")
            nc.gpsimd.dma_start(idt[:], idp[:])
            ones = wpool.tile([1, 512], F16, tag="ones", name="ones")
            nc.vector.memset(ones[:], 1.0)

            def act(dst, src, w, bias, which, tmp_tag):
                """bias + LeakyReLU from PSUM src into SBUF dst.

                "dve" mode expects the bias already accumulated into the
                PSUM tile (via a K=1 ones-matmul) and does mul+max on DVE.
                """
                if cfg[which] == "scalar":
                    nc.scalar.activation(dst, src, PR, bias=bias,
                                         scale=1.0, alpha=ALPHA)
                else:
                    t = xpool.tile([D_OUT, 512], F16, tag=tmp_tag,
                                   name=tmp_tag)
                    nc.vector.tensor_scalar_mul(t[:, :w], src, ALPHA)
                    nc.vector.tensor_max(dst, t[:, :w], src)

            def emit_front(k):
                """DMA in, transpose, L1, L2 -> returns tiles for the back."""
                base = HC - H + k * C        # xin row of chunk col 0
                xraw = xpool.tile([128, NT, D_IN], F16, tag="xraw",
                                  name="xraw")
                nc.sync.dma_start(
                    xraw[:],
                    xin[base:base + CH, :].rearrange(
                        "(t p) f -> p t f", p=128, t=NT))
                xT = xpool.tile([D_IN, CH], F16, tag="xT", name="xT")
                cpeng = ({"dve": nc.vector, "pool": nc.gpsimd}[cfg["copies"]]
                         if cfg["copies"] != "scalar" else None)
                for j in range(NT // 2):
                    psx = pst.tile([128, 128], F16, tag="psx", name="psx")
                    nc.tensor.transpose(
                        psx[:], xraw[:, 2 * j:2 * j + 2, :].rearrange(
                            "p t f -> p (t f)"), idt[:])
                    for h in range(2):
                        dst = xT[:, 256 * j + 128 * h:256 * j + 128 * h + 128]
                        src = psx[D_IN * h:D_IN * (h + 1), :]
                        if cpeng is None:
                            nc.scalar.activation(dst, src, CP)
                        else:
                            cpeng.tensor_copy(dst, src)

                # mask (DMA broadcast [1,CH+1] -> [128,CH+1]); the bwd
                # mask is the same data shifted by one column
                mft = mpool.tile([128, CH + 1], F16, tag="mft", name="mft")
                nc.sync.dma_start(
                    mft[:],
                    mfp[:, base:base + CH + 1].broadcast_to([128, CH + 1]))

                a2 = apool.tile([D_OUT, CH], F16, tag="a2", name="a2")
                W12 = cfg.get("w12", 512)       # L1/L2 psum tile width
                for off in range(0, CH, W12):
                    w = min(W12, CH - off)
                    u1 = psf.tile([D_OUT, W12], F32, tag="u", name="u1")
                    for o2 in range(0, w, 512):
                        w2 = min(512, w - o2)
                        nc.tensor.matmul(u1[:, o2:o2 + w2], wt["w11"][:],
                                         xT[:, off + o2:off + o2 + w2],
                                         start=True, stop=True)
                    a1 = xpool.tile([D_OUT, W12], F16, tag="a1", name="a1")
                    nc.scalar.activation(a1[:, :w], u1[:, :w], PR,
                                         bias=bt["b11"][:], scale=1.0,
                                         alpha=ALPHA)
                    u2 = psf.tile([D_OUT, W12], F32, tag="u", name="u2")
                    for o2 in range(0, w, 512):
                        w2 = min(512, w - o2)
                        nc.tensor.matmul(u2[:, o2:o2 + w2], wt["w12"][:],
                                         a1[:, o2:o2 + w2],
                                         start=True, stop=True)
                    nc.scalar.activation(a2[:, off:off + w], u2[:, :w], PR,
                                         bias=bt["b12"][:], scale=1.0,
                                         alpha=ALPHA)
                return dict(a2=a2, mft=mft)

            def emit_back(k, ft):
                a2, mft = ft["a2"], ft["mft"]
                # segmented max: fwd + bwd scans, pooled broadcast
                fwd = apool.tile([D_OUT, CH], F16, tag="fwd", name="fwd")
                nc.vector.tensor_tensor_scan(
                    fwd[:], mft[:, :CH], a2[:], NEG, op0=ADD, op1=MAX)
                bwd = apool.tile([D_OUT, CH], F16, tag="bwd", name="bwd")
                nc.vector.tensor_tensor_scan(
                    bwd[:, ::-1], mft[:, 1:CH + 1][:, ::-1], a2[:, ::-1],
                    NEG, op0=ADD, op1=MAX)
                pooled = apool.tile([D_OUT, C], F16, tag="pooled",
                                    name="pooled")
                nc.vector.tensor_max(
                    pooled[:], fwd[:, H:H + C], bwd[:, H:H + C])

                # set MLP on pooled (per-element broadcast already) + L31
                a4 = apool.tile([D_OUT, C], F16, tag="a4", name="a4")
                a5 = apool.tile([D_OUT, C], F16, tag="a5", name="a5")
                def bias_mm(u, w, n, last):
                    """accumulate per-feature bias via K=1 ones-matmul"""
                    nc.tensor.matmul(u[:, :w], brt[n][:], ones[:, :w],
                                     start=False, stop=last)

                dve2 = cfg["act2"] == "dve"
                dve31 = cfg["act31"] == "dve"
                for off in range(0, C, 512):
                    w = min(512, C - off)
                    u3 = psb.tile([D_OUT, 512], F32, tag="u", name="u3")
                    nc.tensor.matmul(u3[:, :w], wt["w21"][:],
                                     pooled[:, off:off + w],
                                     start=True, stop=not dve2)
                    if dve2:
                        bias_mm(u3, w, "b21", True)
                    a3 = xpool.tile([D_OUT, 512], F16, tag="a3", name="a3")
                    act(a3[:, :w], u3[:, :w], w, bt["b21"][:], "act2", "t3")
                    u4 = psb.tile([D_OUT, 512], F32, tag="u", name="u4")
                    nc.tensor.matmul(u4[:, :w], wt["w22"][:], a3[:, :w],
                                     start=True, stop=not dve2)
                    if dve2:
                        bias_mm(u4, w, "b22", True)
                    act(a4[:, off:off + w], u4[:, :w], w, bt["b22"][:],
                        "act2", "t4")

                    u5 = psb.tile([D_OUT, 512], F32, tag="u", name="u5")
                    nc.tensor.matmul(u5[:, :w], wt["w31a"][:],
                                     a2[:, H + off:H + off + w],
                                     start=True, stop=False)
                    nc.tensor.matmul(u5[:, :w], wt["w31b"][:],
                                     a4[:, off:off + w],
                                     start=False, stop=not dve31)
                    if dve31:
                        bias_mm(u5, w, "b31", True)
                    act(a5[:, off:off + w], u5[:, :w], w, bt["b31"][:],
                        "act31", "t5")

                # L4: stationary-swap -> element-major out + DMA
                outsb = opool.tile([ET, NE, D_OUT], F32, tag="outsb",
                                   name="outsb")
                for b in range(NE // 4):
                    po = psoo.tile([ET, 4, D_OUT], F32, tag="po", name="po")
                    for t in range(4):
                        e0 = (4 * b + t) * ET
                        nc.tensor.matmul(po[:, t, :],
                                         a5[:, e0:e0 + ET], wt["w32"][:],
                                         start=True, stop=False)
                        nc.tensor.matmul(po[:, t, :], ones[:, :ET], b32t[:],
                                         start=False, stop=True)
                    if cfg.get("act4", "scalar") == "scalar":
                        nc.scalar.activation(outsb[:, 4 * b:4 * b + 4, :],
                                             po[:], PR, bias=0.0, scale=1.0,
                                             alpha=ALPHA)
                    else:
                        t4e = opool.tile([ET, 4, D_OUT], F32, tag="t4e",
                                         name="t4e")
                        nc.vector.tensor_scalar_mul(t4e[:], po[:], ALPHA)
                        nc.vector.tensor_max(outsb[:, 4 * b:4 * b + 4, :],
                                             t4e[:], po[:])
                odma = nc.scalar if cfg["odma"] == "scalar" else nc.sync
                odma.dma_start(
                    out[k * C:(k + 1) * C, :].rearrange(
                        "(t p) f -> p t f", p=ET, t=NE), outsb[:])

            import contextlib
            loop_ctx = (tc.For_i(0, loop_n, 1) if loop_n > 1
                        else contextlib.nullcontext())
            with loop_ctx:
                depth = int(cfg["pipe"])
                if depth:
                    # software pipeline: front(k+depth) is emitted before
                    # back(k) so in-order engine queues overlap chunks
                    fts = {}
                    for k in range(NCH + depth):
                        if k < NCH:
                            fts[k] = emit_front(k)
                        if k >= depth:
                            emit_back(k - depth, fts.pop(k - depth))
                else:
                    for k in range(NCH):
                        emit_back(k, emit_front(k))

    nc.finalize()
    return nc


# ----------------------------------------------------------------------------
# Host side
# ----------------------------------------------------------------------------

_NC_CACHE = {}
_MASK_CACHE = {}
_XBUF_CACHE = {}
_IDENT = np.eye(128, dtype=np.float16)


def _layout(csr_idx):
    csr = np.ascontiguousarray(np.asarray(csr_idx, dtype=np.int64))
    key = (zlib.crc32(csr.tobytes()), csr.shape[0])
    hit = _MASK_CACHE.get(key)
    if hit is not None:
        return hit
    assert csr[0] == 0 and csr[-1] == N
    lmax = int(np.diff(csr).max())
    H = _pick_halo(lmax)
    HC = max(512, H)
    glen = N + 2 * HC + 1
    # single mask: -BIG at every segment start (and all pad positions);
    # the bwd-scan mask is this same array shifted by one column
    mf = np.zeros(glen, np.float16)
    mf[:HC] = NEG
    mf[HC + N:] = NEG
    mf[HC + csr] = NEG
    hit = dict(H=H, HC=HC, mf=mf)
    _MASK_CACHE[key] = hit
    return hit


def prepare(x, csr_idx, w11, s11, b11, w12, s12, b12,
            w21, s21, b21, w22, s22, b22,
            w31, s31, b31, w32, s32, b32, loop_n=1):
    x = np.asarray(x)
    lay = _layout(csr_idx)
    H, HC = lay["H"], lay["HC"]
    EIN = OWN + 2 * HC

    key = (H, HC, loop_n, tuple(sorted(CFG.items())))
    if key not in _NC_CACHE:
        _NC_CACHE[key] = build_nc(H, HC, loop_n=loop_n)
    nc = _NC_CACHE[key]

    # padded fp16 copy of x (cast-assign writes straight into the pad buffer)
    xb = _XBUF_CACHE.get(HC)
    if xb is None:
        xb = np.zeros((N + 2 * HC, D_IN), np.float16)
        _XBUF_CACHE[HC] = xb
    xb[HC:HC + N] = x

    def wprep(w, s):
        return (np.asarray(w) * np.asarray(s)[None, :]).astype(np.float16)

    w31f = wprep(w31, s31)
    params = {
        "w11": wprep(w11, s11), "w12": wprep(w12, s12),
        "w21": wprep(w21, s21), "w22": wprep(w22, s22),
        "w31a": np.ascontiguousarray(w31f[:D_OUT]),
        "w31b": np.ascontiguousarray(w31f[D_OUT:]),
        "w32": wprep(w32, s32),
        "b11": np.asarray(b11, np.float32).reshape(D_OUT, 1),
        "b12": np.asarray(b12, np.float32).reshape(D_OUT, 1),
        "b21": np.asarray(b21, np.float32).reshape(D_OUT, 1),
        "b22": np.asarray(b22, np.float32).reshape(D_OUT, 1),
        "b31": np.asarray(b31, np.float32).reshape(D_OUT, 1),
        "b32r": np.asarray(b32, np.float16).reshape(1, D_OUT),
        "b21r": np.asarray(b21, np.float16).reshape(1, D_OUT),
        "b22r": np.asarray(b22, np.float16).reshape(1, D_OUT),
        "b31r": np.asarray(b31, np.float16).reshape(1, D_OUT),
        "ident": _IDENT,
    }
    in_maps = []
    for c in range(N_CORES):
        lo = c * OWN
        in_maps.append({
            "xin": xb[lo:lo + EIN],
            "mf": lay["mf"][lo:lo + EIN + 1].reshape(1, EIN + 1),
            **params,
        })
    return nc, in_maps, lay


def run_device(nc, in_maps):
    return run_bass_kernel_spmd(nc, in_maps, list(range(N_CORES)))


def postprocess(res, lay, n):
    views = [res.results[c]["out"] for c in range(N_CORES)]
    # the per-core results are views into one (N, D_OUT) host buffer the
    # runtime already assembled -- recover it zero-copy when possible
    b = views[0]
    while getattr(b, "base", None) is not None:
        b = b.base
    try:
        full = b.reshape(N, D_OUT) if b.size == N * D_OUT else None
    except Exception:
        full = None
    if full is not None and full.dtype == np.float32:
        from numpy.lib.array_utils import byte_bounds
        lo0 = byte_bounds(full)[0]
        ok = all(
            byte_bounds(views[c])[0] == lo0 + c * OWN * D_OUT * 4
            and views[c].shape == (OWN, D_OUT)
            for c in range(N_CORES))
        if ok:
            return full
    return np.concatenate(views, axis=0)


def kernel(x, csr_idx, **kw):
    x = np.asarray(x)
    nc, in_maps, lay = prepare(x, csr_idx, **kw)
    res = run_device(nc, in_maps)
    return postprocess(res, lay, x.shape[0])
